# revision 2
# baseline (speedup 1.0000x reference)
"""TRN2 Bass kernel for nn_EnsemblePointNet: 1296 independent 4-layer MLPs.

Strategy: shard the model dim (1296 -> 162 per core) across 8 NeuronCores.
The kernel is PSUM-eviction-bound on TRN2 (matmul output must be fp32 in
PSUM; each of the 3 hidden activations must cross the 32-bit/cycle PSUM
read ports of ACT+DVE). Design choices:

- All matmul operands fp16 (full PE rate, FWL-eligible weight loads).
- L0 bias folded into the matmul via K augmentation (K=8 -> 9 with a ones
  row in x and the bias row in W0), so h1 evictions are bias-free relu.
- L0 pairs of models run as concurrent row-tiled matmuls at tile positions
  (0,0)/(32,0) (K=9 <= 32).
- L3 (128 -> 1) packs 4 models as concurrent col-tiled matmuls at
  (0, 32j) with M=32; output bias b3 is partition-replicated so the whole
  quad's output eviction is one instruction; result rows DMA straight to y.
- Evictions are whole-[128,1024] instructions balanced ACT/DVE (7/6 per
  quad), fusing bias+relu+fp16-cast in one pass.
"""

import sys

sys.path.insert(0, "/opt/trn_rl_repo")

import numpy as np

import concourse.bass as bass
import concourse.mybir as mybir
import concourse.tile as tile
from concourse import bacc
from concourse.bass_utils import run_bass_kernel_spmd

F32 = mybir.dt.float32
F16 = mybir.dt.float16
AF = mybir.ActivationFunctionType
OP = mybir.AluOpType

M_TOT = 1296
N_CORES = 8
M_LOC = M_TOT // N_CORES  # 162
B = 1024
DIN = 8
KA = DIN + 1  # augmented contraction (bias row)
H = 128
HB = 512

QUAD = 4
M_PAD = 164   # 41 quads; models 162,163 are zero dummies
NQ = M_PAD // QUAD   # 41
NPAIR = M_PAD // 2   # 82; pair 81 is all-dummy


def build_nc(m_loc=M_LOC, loop_n=1):
    assert m_loc == M_LOC
    m_pad, nq, npair = M_PAD, NQ, NPAIR
    nc = bacc.Bacc("TRN2", target_bir_lowering=False, debug=False)
    xp = nc.dram_tensor("xp", [npair, 41, B], F16, kind="ExternalInput").ap()
    w0p = nc.dram_tensor("w0p", [npair, 41, H], F16, kind="ExternalInput").ap()
    w12 = nc.dram_tensor("w12", [nq, H, 8 * H], F16, kind="ExternalInput").ap()
    w3t = nc.dram_tensor("w3t", [H, m_pad + 31], F16, kind="ExternalInput").ap()
    b1t = nc.dram_tensor("b1t", [H, m_pad], F32, kind="ExternalInput").ap()
    b2t = nc.dram_tensor("b2t", [H, m_pad], F32, kind="ExternalInput").ap()
    b3r = nc.dram_tensor("b3r", [H, nq], F32, kind="ExternalInput").ap()
    y = nc.dram_tensor("y", [m_pad, B], F32, kind="ExternalOutput").ap()

    with tile.TileContext(nc) as tc:
        with (
            tc.tile_pool(name="consts", bufs=1) as consts,
            tc.tile_pool(name="wpool", bufs=3) as wpool,
            tc.tile_pool(name="w0pool", bufs=4) as w0pool,
            tc.tile_pool(name="xpool", bufs=4) as xpool,
            tc.tile_pool(name="hpool", bufs=10) as hpool,
            tc.tile_pool(name="spool", bufs=2) as spool,
            tc.tile_pool(name="zpool", bufs=4, space="PSUM") as zpool,
        ):
            w3t_s = consts.tile([H, m_pad + 31], F16)
            nc.sync.dma_start(out=w3t_s, in_=w3t)
            b1t_s = consts.tile([H, m_pad], F32)
            nc.sync.dma_start(out=b1t_s, in_=b1t)
            b2t_s = consts.tile([H, m_pad], F32)
            nc.sync.dma_start(out=b2t_s, in_=b2t)
            b3r_s = consts.tile([H, nq], F32)
            nc.sync.dma_start(out=b3r_s, in_=b3r)

            def evict(i, dst, z, bias_ap):
                # dst[128,1024] sbuf <- relu(z[128,1024] psum (+ bias))
                # alternate models between ACT (even) and DVE (odd)
                if i % 2 == 0:
                    b = 0.0 if bias_ap is None else bias_ap
                    nc.scalar.activation(dst, z, AF.Relu, bias=b, scale=1.0)
                else:
                    if bias_ap is None:
                        nc.vector.tensor_scalar(
                            out=dst, in0=z, scalar1=0.0, scalar2=None, op0=OP.max
                        )
                    else:
                        nc.vector.tensor_scalar(
                            out=dst, in0=z, scalar1=bias_ap, scalar2=0.0,
                            op0=OP.add, op1=OP.max,
                        )

            def body():
                for q in range(nq):
                    m0 = q * QUAD
                    w12s = wpool.tile([H, 8 * H], F16, tag="w12")
                    nc.sync.dma_start(out=w12s, in_=w12[q])
                    xts, w0ts = [], []
                    for p in range(2):
                        xt_ = xpool.tile([H, B], F16, tag="xp")
                        nc.sync.dma_start(out=xt_[0:41, :], in_=xp[2 * q + p])
                        xts.append(xt_)
                        w0_ = w0pool.tile([H, H], F16, tag="w0")
                        nc.sync.dma_start(out=w0_[0:41, :], in_=w0p[2 * q + p])
                        w0ts.append(w0_)

                    # ---- L0: pair-row-tiled concurrent matmuls (K=9) ----
                    z0 = []
                    for p in range(2):
                        za = zpool.tile([H, B], F32, tag="z")
                        zb = zpool.tile([H, B], F32, tag="z")
                        for hb in range(2):
                            sl = slice(hb * HB, (hb + 1) * HB)
                            nc.tensor.matmul(
                                za[:, sl], w0ts[p][0:KA, :], xts[p][0:KA, sl],
                                start=True, stop=True, tile_position=(0, 0),
                            )
                            nc.tensor.matmul(
                                zb[:, sl], w0ts[p][32 : 32 + KA, :],
                                xts[p][32 : 32 + KA, sl],
                                start=True, stop=True, tile_position=(32, 0),
                            )
                        z0 += [za, zb]
                    h1 = []
                    for i in range(QUAD):
                        ht = hpool.tile([H, B], F16, tag="h")
                        evict(i, ht, z0[i], None)
                        h1.append(ht)

                    # ---- L1 / L2: full 128x128 matmuls, bias in eviction ----
                    def layer(hs, lsel, bias_t):
                        zs = []
                        for i in range(QUAD):
                            zt = zpool.tile([H, B], F32, tag="z")
                            lhs = w12s[:, (2 * i + lsel) * H : (2 * i + lsel + 1) * H]
                            for hb in range(2):
                                sl = slice(hb * HB, (hb + 1) * HB)
                                nc.tensor.matmul(
                                    zt[:, sl], lhs, hs[i][:, sl],
                                    start=True, stop=True,
                                )
                            zs.append(zt)
                        outs = []
                        for i in range(QUAD):
                            ht = hpool.tile([H, B], F16, tag="h")
                            evict(i, ht, zs[i], bias_t[:, m0 + i : m0 + i + 1])
                            outs.append(ht)
                        return outs

                    h2 = layer(h1, 0, b1t_s)
                    h3 = layer(h2, 1, b2t_s)

                    # ---- L3: col-tiled quad (M=32 each), bias replicated ----
                    zq = zpool.tile([H, B], F32, tag="z")
                    for hb in range(2):
                        sl = slice(hb * HB, (hb + 1) * HB)
                        for j in range(QUAD):
                            nc.tensor.matmul(
                                zq[32 * j : 32 * j + 32, sl],
                                w3t_s[:, m0 + j : m0 + j + 32],
                                h3[j][:, sl],
                                start=True, stop=True, tile_position=(0, 32 * j),
                            )
                    scr = spool.tile([H, B], F32, tag="scr")
                    nc.scalar.add(scr, zq, b3r_s[:, q : q + 1])
                    sv = scr.rearrange("(a p) b -> a p b", a=4)[:, 0, :]
                    nc.sync.dma_start(out=y[m0 : m0 + QUAD, :], in_=sv)

            if loop_n > 1:
                with tc.For_i(0, loop_n, 1):
                    body()
            else:
                body()

    nc.compile()
    return nc


_NC_CACHE = {}


def _get_nc(m_loc):
    if m_loc not in _NC_CACHE:
        _NC_CACHE[m_loc] = build_nc(m_loc)
    return _NC_CACHE[m_loc]


def _prep_core_inputs(x, W0, b0, W1, b1, W2, b2, W3, b3, sl):
    m_loc = sl.stop - sl.start
    assert m_loc == M_LOC
    xt = np.transpose(x[sl], (0, 2, 1)).astype(np.float16)  # [162, 8, B]

    xp = np.zeros((NPAIR, 41, B), np.float16)
    xp[:81, 0:8, :] = xt[0::2]
    xp[:81, 8, :] = 1.0
    xp[:81, 32:40, :] = xt[1::2]
    xp[:81, 40, :] = 1.0

    w0p = np.zeros((NPAIR, 41, H), np.float16)
    w0p[:81, 0:8, :] = W0[sl][0::2]
    w0p[:81, 8, :] = b0[sl][0::2]
    w0p[:81, 32:40, :] = W0[sl][1::2]
    w0p[:81, 40, :] = b0[sl][1::2]

    # w12[q, h, (2i+l)*H + k] = W_{l+1}[4q+i, h, k]
    a = np.zeros((M_PAD, 2, H, H), np.float16)
    a[:m_loc, 0] = W1[sl]
    a[:m_loc, 1] = W2[sl]
    w12 = np.ascontiguousarray(
        a.reshape(NQ, QUAD, 2, H, H).transpose(0, 3, 1, 2, 4).reshape(NQ, H, 8 * H)
    )

    w3tp = np.zeros((H, M_PAD + 31), np.float16)
    w3tp[:, :m_loc] = W3[sl, :, 0].T.astype(np.float16)

    b1t = np.zeros((H, M_PAD), np.float32)
    b1t[:, :m_loc] = b1[sl].T
    b2t = np.zeros((H, M_PAD), np.float32)
    b2t[:, :m_loc] = b2[sl].T

    tmp = np.zeros((NQ, QUAD), np.float32)
    tmp.reshape(-1)[:m_loc] = b3[sl, 0]
    b3r = np.repeat(tmp.T, 32, axis=0)  # [128, NQ]

    return {
        "xp": xp,
        "w0p": w0p,
        "w12": w12,
        "w3t": w3tp,
        "b1t": np.ascontiguousarray(b1t),
        "b2t": np.ascontiguousarray(b2t),
        "b3r": np.ascontiguousarray(b3r),
    }


def kernel(x, W0, b0, W1, b1, W2, b2, W3, b3):
    x = np.asarray(x, dtype=np.float32)
    W0 = np.asarray(W0, np.float32); b0 = np.asarray(b0, np.float32)
    W1 = np.asarray(W1, np.float32); b1 = np.asarray(b1, np.float32)
    W2 = np.asarray(W2, np.float32); b2 = np.asarray(b2, np.float32)
    W3 = np.asarray(W3, np.float32); b3 = np.asarray(b3, np.float32)

    m_tot = x.shape[0]
    m_loc = m_tot // N_CORES
    nc = _get_nc(m_loc)
    in_maps = [
        _prep_core_inputs(x, W0, b0, W1, b1, W2, b2, W3, b3,
                          slice(c * m_loc, (c + 1) * m_loc))
        for c in range(N_CORES)
    ]
    res = run_bass_kernel_spmd(nc, in_maps, core_ids=list(range(N_CORES)))
    out = np.concatenate([r["y"][:m_loc] for r in res.results], axis=0)
    return out.reshape(m_tot, B, 1).astype(np.float32)


# revision 6
# speedup vs baseline: 1.0869x; 1.0869x over previous
"""TRN2 Bass kernel for nn_EnsemblePointNet: 1296 independent 4-layer MLPs.

Strategy: shard the model dim (1296 -> 162 per core) across 8 NeuronCores.
The kernel is PSUM-eviction-bound on TRN2 (matmul output must be fp32 in
PSUM; each of the 3 hidden activations must cross the 32-bit/cycle PSUM
read ports of ACT+DVE). Design choices:

- All matmul operands fp16 (full PE rate, FWL-eligible weight loads).
- L0 bias folded into the matmul via K augmentation (K=8 -> 9 with a ones
  row in x and the bias row in W0), so h1 evictions are bias-free relu.
- L0 pairs of models run as concurrent row-tiled matmuls at tile positions
  (0,0)/(32,0) (K=9 <= 32).
- L3 (128 -> 1) packs 4 models as concurrent col-tiled matmuls at
  (0, 32j) with M=32; output bias b3 is partition-replicated so the whole
  quad's output eviction is one instruction; result rows DMA straight to y.
- Evictions are whole-[128,1024] instructions balanced ACT/DVE (7/6 per
  quad), fusing bias+relu+fp16-cast in one pass.
"""

import sys

sys.path.insert(0, "/opt/trn_rl_repo")

import numpy as np

import concourse.bass as bass
import concourse.mybir as mybir
import concourse.tile as tile
from concourse import bacc
from concourse.bass_utils import run_bass_kernel_spmd

F32 = mybir.dt.float32
F16 = mybir.dt.float16
AF = mybir.ActivationFunctionType
OP = mybir.AluOpType

M_TOT = 1296
N_CORES = 8
M_LOC = M_TOT // N_CORES  # 162
B = 1024
DIN = 8
KA = DIN + 1  # augmented contraction (bias row)
H = 128
HB = 512

QUAD = 4
M_PAD = 164   # 41 quads; models 162,163 are zero dummies
NQ = M_PAD // QUAD   # 41
NPAIR = M_PAD // 2   # 82; pair 81 is all-dummy


def build_nc(m_loc=M_LOC, loop_n=1):
    assert m_loc == M_LOC
    m_pad, nq, npair = M_PAD, NQ, NPAIR
    nc = bacc.Bacc("TRN2", target_bir_lowering=False, debug=False)
    xp = nc.dram_tensor("xp", [npair, 41, B], F16, kind="ExternalInput").ap()
    w0p = nc.dram_tensor("w0p", [npair, 41, H], F16, kind="ExternalInput").ap()
    w12 = nc.dram_tensor("w12", [nq, H, 8 * H], F16, kind="ExternalInput").ap()
    w3t = nc.dram_tensor("w3t", [H, m_pad + 31], F16, kind="ExternalInput").ap()
    b1t = nc.dram_tensor("b1t", [H, m_pad], F32, kind="ExternalInput").ap()
    b2t = nc.dram_tensor("b2t", [H, m_pad], F32, kind="ExternalInput").ap()
    b3r = nc.dram_tensor("b3r", [H, nq], F32, kind="ExternalInput").ap()
    y = nc.dram_tensor("y", [m_pad, B], F32, kind="ExternalOutput").ap()

    with tile.TileContext(nc) as tc:
        with (
            tc.tile_pool(name="consts", bufs=1) as consts,
            tc.tile_pool(name="wpool", bufs=4) as wpool,
            tc.tile_pool(name="w0pool", bufs=6) as w0pool,
            tc.tile_pool(name="xpool", bufs=5) as xpool,
            tc.tile_pool(name="hpool", bufs=16) as hpool,
            tc.tile_pool(name="spool", bufs=2) as spool,
            tc.tile_pool(name="zpool", bufs=4, space="PSUM") as zpool,
        ):
            w3t_s = consts.tile([H, m_pad + 31], F16)
            nc.sync.dma_start(out=w3t_s, in_=w3t)
            b1t_s = consts.tile([H, m_pad], F32)
            nc.sync.dma_start(out=b1t_s, in_=b1t)
            b2t_s = consts.tile([H, m_pad], F32)
            nc.sync.dma_start(out=b2t_s, in_=b2t)
            b3r_s = consts.tile([H, nq], F32)
            nc.sync.dma_start(out=b3r_s, in_=b3r)

            def evict(i, dst, z, bias_ap):
                # dst[128,1024] sbuf <- relu(z[128,1024] psum (+ bias))
                # alternate models between ACT (even) and DVE (odd)
                if i % 2 == 0:
                    b = 0.0 if bias_ap is None else bias_ap
                    nc.scalar.activation(dst, z, AF.Relu, bias=b, scale=1.0)
                else:
                    if bias_ap is None:
                        nc.vector.tensor_scalar(
                            out=dst, in0=z, scalar1=0.0, scalar2=None, op0=OP.max
                        )
                    else:
                        nc.vector.tensor_scalar(
                            out=dst, in0=z, scalar1=bias_ap, scalar2=0.0,
                            op0=OP.add, op1=OP.max,
                        )

            def emit_l3(m0p, qp, h3p):
                # deferred L3 for quad qp: col-tiled matmuls + bias-add
                # eviction + direct y DMA
                zqt = zpool.tile([H, B], F32, tag="z")
                for hb in range(2):
                    sl = slice(hb * HB, (hb + 1) * HB)
                    for j in range(QUAD):
                        nc.tensor.matmul(
                            zqt[32 * j : 32 * j + 32, sl],
                            w3t_s[:, m0p + j : m0p + j + 32],
                            h3p[j][:, sl],
                            start=True, stop=True, tile_position=(0, 32 * j),
                        )
                scr = spool.tile([H, B], F32, tag="scr")
                nc.scalar.add(scr, zqt, b3r_s[:, qp : qp + 1])
                sv = scr.rearrange("(a p) b -> a p b", a=4)[:, 0, :]
                nc.sync.dma_start(out=y[m0p : m0p + QUAD, :], in_=sv)

            def body():
                pending = None
                for q in range(nq):
                    m0 = q * QUAD
                    w12s = wpool.tile([H, 8 * H], F16, tag="w12")
                    nc.sync.dma_start(out=w12s, in_=w12[q])
                    xts, w0ts = [], []
                    for p in range(2):
                        xt_ = xpool.tile([H, B], F16, tag="xp")
                        nc.sync.dma_start(out=xt_[0:41, :], in_=xp[2 * q + p])
                        xts.append(xt_)
                        w0_ = w0pool.tile([H, H], F16, tag="w0")
                        nc.sync.dma_start(out=w0_[0:41, :], in_=w0p[2 * q + p])
                        w0ts.append(w0_)

                    if pending is not None:
                        emit_l3(*pending)
                        pending = None

                    # ---- L0: pair-row-tiled concurrent matmuls (K=9) ----
                    z0 = []
                    for p in range(2):
                        za = zpool.tile([H, B], F32, tag="z")
                        zb = zpool.tile([H, B], F32, tag="z")
                        for hb in range(2):
                            sl = slice(hb * HB, (hb + 1) * HB)
                            nc.tensor.matmul(
                                za[:, sl], w0ts[p][0:KA, :], xts[p][0:KA, sl],
                                start=True, stop=True, tile_position=(0, 0),
                            )
                            nc.tensor.matmul(
                                zb[:, sl], w0ts[p][32 : 32 + KA, :],
                                xts[p][32 : 32 + KA, sl],
                                start=True, stop=True, tile_position=(32, 0),
                            )
                        z0 += [za, zb]
                    h1 = []
                    for i in range(QUAD):
                        ht = hpool.tile([H, B], F16, tag="h")
                        evict(i, ht, z0[i], None)
                        h1.append(ht)

                    # ---- L1 / L2: full 128x128 matmuls, bias in eviction ----
                    def layer(hs, lsel, bias_t):
                        zs = []
                        for i in range(QUAD):
                            zt = zpool.tile([H, B], F32, tag="z")
                            lhs = w12s[:, (2 * i + lsel) * H : (2 * i + lsel + 1) * H]
                            for hb in range(2):
                                sl = slice(hb * HB, (hb + 1) * HB)
                                nc.tensor.matmul(
                                    zt[:, sl], lhs, hs[i][:, sl],
                                    start=True, stop=True,
                                )
                            zs.append(zt)
                        outs = []
                        for i in range(QUAD):
                            ht = hpool.tile([H, B], F16, tag="h")
                            evict(i, ht, zs[i], bias_t[:, m0 + i : m0 + i + 1])
                            outs.append(ht)
                        return outs

                    h2 = layer(h1, 0, b1t_s)
                    h3 = layer(h2, 1, b2t_s)
                    pending = (m0, q, h3)

                if pending is not None:
                    emit_l3(*pending)
                    pending = None

            if loop_n > 1:
                with tc.For_i(0, loop_n, 1):
                    body()
            else:
                body()

    nc.compile()
    return nc


_NC_CACHE = {}


def _get_nc(m_loc):
    if m_loc not in _NC_CACHE:
        _NC_CACHE[m_loc] = build_nc(m_loc)
    return _NC_CACHE[m_loc]


def _prep_core_inputs(x, W0, b0, W1, b1, W2, b2, W3, b3, sl):
    m_loc = sl.stop - sl.start
    assert m_loc == M_LOC
    xt = np.transpose(x[sl], (0, 2, 1)).astype(np.float16)  # [162, 8, B]

    xp = np.zeros((NPAIR, 41, B), np.float16)
    xp[:81, 0:8, :] = xt[0::2]
    xp[:81, 8, :] = 1.0
    xp[:81, 32:40, :] = xt[1::2]
    xp[:81, 40, :] = 1.0

    w0p = np.zeros((NPAIR, 41, H), np.float16)
    w0p[:81, 0:8, :] = W0[sl][0::2]
    w0p[:81, 8, :] = b0[sl][0::2]
    w0p[:81, 32:40, :] = W0[sl][1::2]
    w0p[:81, 40, :] = b0[sl][1::2]

    # w12[q, h, (2i+l)*H + k] = W_{l+1}[4q+i, h, k]
    a = np.zeros((M_PAD, 2, H, H), np.float16)
    a[:m_loc, 0] = W1[sl]
    a[:m_loc, 1] = W2[sl]
    w12 = np.ascontiguousarray(
        a.reshape(NQ, QUAD, 2, H, H).transpose(0, 3, 1, 2, 4).reshape(NQ, H, 8 * H)
    )

    w3tp = np.zeros((H, M_PAD + 31), np.float16)
    w3tp[:, :m_loc] = W3[sl, :, 0].T.astype(np.float16)

    b1t = np.zeros((H, M_PAD), np.float32)
    b1t[:, :m_loc] = b1[sl].T
    b2t = np.zeros((H, M_PAD), np.float32)
    b2t[:, :m_loc] = b2[sl].T

    tmp = np.zeros((NQ, QUAD), np.float32)
    tmp.reshape(-1)[:m_loc] = b3[sl, 0]
    b3r = np.repeat(tmp.T, 32, axis=0)  # [128, NQ]

    return {
        "xp": xp,
        "w0p": w0p,
        "w12": w12,
        "w3t": w3tp,
        "b1t": np.ascontiguousarray(b1t),
        "b2t": np.ascontiguousarray(b2t),
        "b3r": np.ascontiguousarray(b3r),
    }


def kernel(x, W0, b0, W1, b1, W2, b2, W3, b3):
    x = np.asarray(x, dtype=np.float32)
    W0 = np.asarray(W0, np.float32); b0 = np.asarray(b0, np.float32)
    W1 = np.asarray(W1, np.float32); b1 = np.asarray(b1, np.float32)
    W2 = np.asarray(W2, np.float32); b2 = np.asarray(b2, np.float32)
    W3 = np.asarray(W3, np.float32); b3 = np.asarray(b3, np.float32)

    m_tot = x.shape[0]
    m_loc = m_tot // N_CORES
    nc = _get_nc(m_loc)
    in_maps = [
        _prep_core_inputs(x, W0, b0, W1, b1, W2, b2, W3, b3,
                          slice(c * m_loc, (c + 1) * m_loc))
        for c in range(N_CORES)
    ]
    res = run_bass_kernel_spmd(nc, in_maps, core_ids=list(range(N_CORES)))
    out = np.concatenate([r["y"][:m_loc] for r in res.results], axis=0)
    return out.reshape(m_tot, B, 1).astype(np.float32)
